# revision 8
# baseline (speedup 1.0000x reference)
"""Trainium2 Bass kernel for an 11-stage butterfly linear layer + bias.

Problem: x (16384, 2048) fp32; out[b, :] = B @ x[b, :] + bias where B is the
composition of 11 butterfly stages (strides 1..1024), each an elementwise 2x2
mix of position pairs with learned per-pair coefficients.

Factorization (positions p = blk*128 + w, blk in [0,16), w in [0,128)):
  - Stages 0-6 (strides 1..64) mix within a 128-block -> block-diagonal
    D = diag(D_0..D_15), each 128x128 dense.
  - Stages 7-10 (strides 128..1024) mix across blocks, separately per w ->
    per-w 16x16 matrices C_w.  Grouping q = w8*16 + b over w-group t = w//8
    makes this block-diagonal too (128x128 per group t).

v3 design (vs 185 us baseline / 148 us v2):
  - x pre-transposed on the HOST into per-chunk [pos, block, batch] layout:
    zero TensorE transposes, contiguous input DMA.
  - bf16 output (host casts back to f32): per-core HBM 8 MiB in + 8 MiB out.
  - CH=1024 batch rows per chunk (2 chunks/core): halves the DMA instruction
    count vs CH=512 (dma_start costs ~0.6-2.7us of ring-sequencer time each)
    and doubles permute descriptor size to 2 KiB.
  - Mid permute as 16 per-t SBUF->SBUF DMAs per chunk, alternating SP/ACT
    rings, into per-t Yp tiles (finer deps, less SBUF).
  - MM2 matmuls write STRIDED PSUM APs (out block-major) so the DVE bias-add
    drain reads/writes contiguously (measured 1.2us for the 4D-AP variant).
  - MM1 drains split ACT(3/4) / DVE(1/4); software-pipelined emission keeps
    the PE inside the HAM window.
"""

import sys

import numpy as np

sys.path.insert(0, "/opt/trn_rl_repo")

import concourse.bass as bass  # noqa: E402
import concourse.mybir as mybir  # noqa: E402
import concourse.tile as tile  # noqa: E402
from concourse import bacc  # noqa: E402
from concourse.bass import ds, ts  # noqa: E402
from concourse.bass_utils import run_bass_kernel_spmd  # noqa: E402

N = 2048
LOG_N = 11
NCORES = 8
BATCH = 16384
BPC = BATCH // NCORES  # batch rows per core
P = 128
NB = 16  # number of 128-blocks
CH = 1024  # batch rows per pipeline chunk
CHN = BPC // CH  # chunks per core
HQ = CH // P  # 128-row groups per chunk

WARMUP_MMS = 16  # PE warmup matmuls (N=256) overlapping the first input DMA
STRIDED_PSUM_MM2 = False  # MM2 writes block-major psum -> contiguous DVE add

PROFILE = False
LAST_RESULTS = None

_NC_CACHE = {}


def _emit_body(ctx, tc, aps):
    nc = tc.nc
    x_ap, w1_ap, c2_ap, bb_ap, out_ap = aps
    f32 = mybir.dt.float32
    bf16 = mybir.dt.bfloat16

    const = ctx.enter_context(tc.tile_pool(name="const", bufs=1))
    W1 = const.tile([P, NB * P], bf16)
    C2 = const.tile([P, NB * P], bf16)
    BB = const.tile([P, N], bf16)
    nc.scalar.dma_start(W1[:], w1_ap)
    nc.scalar.dma_start(C2[:], c2_ap)
    nc.scalar.dma_start(BB[:], bb_ap)

    HB = NB // 2  # b-blocks per input half-tile
    xpool = ctx.enter_context(tc.tile_pool(name="xin", bufs=2))
    ypool = ctx.enter_context(tc.tile_pool(name="ymid", bufs=2))
    yppool = ctx.enter_context(tc.tile_pool(name="ypmid", bufs=2 * NB))
    opool = ctx.enter_context(tc.tile_pool(name="oout", bufs=4))
    ps_m1 = ctx.enter_context(tc.tile_pool(name="ps_m1", bufs=2, space="PSUM"))
    ps_m2 = ctx.enter_context(tc.tile_pool(name="ps_m2", bufs=2, space="PSUM"))

    # ---- input DMAs: two half-tiles (8 b-blocks) per chunk ----
    xts = {}
    for c in range(CHN):
        for h in range(2):
            xts[(c, h)] = xpool.tile([P, HB * CH], bf16, name=f"A_{c}_{h}", tag="A")

    def dma_in(c, h):
        nc.sync.dma_start(
            xts[(c, h)][:],
            x_ap[c * P : (c + 1) * P, h * HB * CH : (h + 1) * HB * CH],
        )

    for c in range(CHN):
        for h in range(2):
            dma_in(c, h)

    # ---- PE warmup: get HAM to K=8/8 while chunk 0's DMA lands ----
    wps = ps_m1.tile([P, CH], f32, name="warm", tag="pp")
    for i in range(WARMUP_MMS):
        nc.tensor.matmul(
            wps[:, ds(0, 256)], W1[:, ts(0, P)], C2[:, ds(0, 256)],
            start=True, stop=True,
        )

    ypss = {}

    def mm1_block(c):
        """Per b: 2 matmuls (N=512) into a 2-bank PSUM tile + copy drain."""
        Ysb = ypool.tile([P, NB * CH], bf16, name=f"Ysb_{c}", tag="Ysb")
        for b in range(NB):
            A = xts[(c, b // HB)]
            pp = ps_m1.tile([P, CH], f32, name=f"pp_{c}_{b}", tag="pp")
            for i in range(2):
                nc.tensor.matmul(
                    pp[:, ts(i, CH // 2)],
                    W1[:, ts(b, P)],
                    A[:, ds((b % HB) * CH + i * (CH // 2), CH // 2)],
                    start=True,
                    stop=True,
                )
            if b % 4 == 3:
                nc.vector.tensor_copy(Ysb[:, ts(b, CH)], pp[:])
            else:
                nc.scalar.copy(Ysb[:, ts(b, CH)], pp[:])
        # permute: Yp_t[w8*16+b, f] = Ysb[t*8+w8, b*CH+f]
        yps = []
        for t in range(NB):
            Yp = yppool.tile([P, CH], bf16, name=f"Yp_{c}_{t}", tag="Yp")
            psrc = Ysb[8 * t : 8 * t + 8, :].rearrange("w (b f) -> w b f", b=NB, f=CH)
            eng = nc.sync if t % 2 == 0 else nc.scalar
            eng.dma_start(Yp[:], psrc)
            yps.append(Yp)
        ypss[c] = yps

    def mm2_block(c):
        """Per hh: 16 data-stationary matmuls + bias-add drains + DMA out."""
        yps = ypss[c]
        for hh in range(HQ):
            O = opool.tile([P, N], bf16, name=f"O_{c}_{hh}", tag="O")
            for tp in range(2):
                pz = ps_m2.tile([P, 8 * P], f32, name=f"pz_{c}_{hh}_{tp}", tag="pz")
                for j in range(8):
                    t = tp * 8 + j
                    if STRIDED_PSUM_MM2:
                        # col (bo, wo8) -> pz col bo*64 + j*8 + wo8 (block-major)
                        dst = pz[:].rearrange(
                            "p (b t w) -> p t b w", b=16, t=8, w=8
                        )[:, j]
                    else:
                        dst = pz[:, ts(j, P)]
                    nc.tensor.matmul(
                        dst,
                        yps[t][:, ts(hh, P)],
                        C2[:, ts(t, P)],
                        start=True,
                        stop=True,
                    )
                if STRIDED_PSUM_MM2:
                    # O col n = bo*128 + tp*64 + v, v = j*8+wo8 contiguous 64
                    dsto = O[:].rearrange("p (b u) -> p b u", b=16, u=128)[
                        :, :, tp * 64 : (tp + 1) * 64
                    ]
                    src = pz[:].rearrange("p (b v) -> p b v", b=16, v=64)
                    bsrc = BB[:].rearrange("p (b u) -> p b u", b=16, u=128)[
                        :, :, tp * 64 : (tp + 1) * 64
                    ]
                else:
                    dsto = O[:].rearrange("p (b t w) -> p b t w", b=16, t=16, w=8)[
                        :, :, tp * 8 : (tp + 1) * 8, :
                    ]
                    src = pz[:].rearrange("p (t b w) -> p b t w", t=8, b=16, w=8)
                    bsrc = BB[:].rearrange("p (b t w) -> p b t w", b=16, t=16, w=8)[
                        :, :, tp * 8 : (tp + 1) * 8, :
                    ]
                nc.vector.tensor_add(dsto, src, bsrc)
            nc.sync.dma_start(
                out_ap[c * CH + hh * P : c * CH + (hh + 1) * P, :], O[:]
            )

    for c in range(CHN):
        mm1_block(c)
        if c >= 1:
            mm2_block(c - 1)
    mm2_block(CHN - 1)


def build_nc():
    nc = bacc.Bacc(
        "TRN2",
        target_bir_lowering=False,
        debug=False,
        num_devices=NCORES,
    )
    x_ap = nc.dram_tensor(
        "x", [CHN * P, NB * CH], mybir.dt.bfloat16, kind="ExternalInput"
    ).ap()
    w1_ap = nc.dram_tensor("w1", [P, NB * P], mybir.dt.bfloat16, kind="ExternalInput").ap()
    c2_ap = nc.dram_tensor("c2", [P, NB * P], mybir.dt.bfloat16, kind="ExternalInput").ap()
    bb_ap = nc.dram_tensor("bb", [P, N], mybir.dt.bfloat16, kind="ExternalInput").ap()
    out_ap = nc.dram_tensor("out", [BPC, N], mybir.dt.bfloat16, kind="ExternalOutput").ap()

    from contextlib import ExitStack

    with tile.TileContext(nc) as tc:
        with ExitStack() as ctx:
            _emit_body(ctx, tc, (x_ap, w1_ap, c2_ap, bb_ap, out_ap))
    nc.compile()
    return nc


def _butterfly_apply(tw, X, idx_lo, idx_hi):
    """Apply butterfly stages [idx_lo, idx_hi) to rows of X. tw: (LOG_N, N//2, 2, 2)."""
    out = X
    for idx in range(idx_lo, idx_hi):
        s = 1 << idx
        g = N // (2 * s)
        T = tw[idx].reshape(g, s, 2, 2)
        xr = out.reshape(-1, g, 2, s)
        out = np.einsum("gsij,bgjs->bgis", T, xr).reshape(-1, N)
    return out


def host_weights(twiddle, bias):
    """Build device constants from the twiddle/bias arrays."""
    import ml_dtypes

    tw = np.asarray(twiddle, dtype=np.float64)[0, 0]  # (LOG_N, N//2, 2, 2)
    eye = np.eye(N, dtype=np.float64)
    R1 = _butterfly_apply(tw, eye, 0, 7)  # = D^T, block-diagonal
    R2 = _butterfly_apply(tw, eye, 7, LOG_N)  # = C^T

    # W1 lhsT per block b: lhsT[p, w] = D_b[w, p] = R1 block (b, b)
    w1 = np.concatenate(
        [R1[b * P : (b + 1) * P, b * P : (b + 1) * P] for b in range(NB)], axis=1
    )
    # C2 lhsT per w-group t: rows q = w8*16+b (mid pos), cols j = bo*8+wo8 (out pos)
    c2 = np.zeros((P, NB * P))
    q = np.arange(P)
    for t in range(NB):
        pm = (q % 16) * P + t * 8 + (q // 16)
        pn = (q // 8) * P + t * 8 + (q % 8)
        c2[:, t * P : (t + 1) * P] = R2[np.ix_(pm, pn)]
    bb = np.broadcast_to(np.asarray(bias, dtype=np.float64)[None, :], (P, N))
    return (
        np.ascontiguousarray(w1.astype(ml_dtypes.bfloat16)),
        np.ascontiguousarray(c2.astype(ml_dtypes.bfloat16)),
        np.ascontiguousarray(bb.astype(ml_dtypes.bfloat16)),
    )


def host_x(x):
    """bf16-cast + per-core chunked transpose: [c][fc][p][b][f] layout."""
    import ml_dtypes

    xb = np.asarray(x).astype(ml_dtypes.bfloat16)
    # rows = c*2048 + fc*CH + f; cols = b*128 + p
    xr = xb.reshape(NCORES, CHN, CH, NB, P).transpose(0, 1, 4, 3, 2)
    return np.ascontiguousarray(xr)  # (8, CHN, 128, 16, CH)


def kernel(x, twiddle, bias):
    global LAST_RESULTS

    assert x.shape == (BATCH, N), x.shape

    if "nc" not in _NC_CACHE:
        _NC_CACHE["nc"] = build_nc()
    nc = _NC_CACHE["nc"]

    w1, c2, bb = host_weights(twiddle, bias)
    xr = host_x(x)
    in_maps = [
        {
            "x": xr[c].reshape(CHN * P, NB * CH),
            "w1": w1,
            "c2": c2,
            "bb": bb,
        }
        for c in range(NCORES)
    ]
    res = run_bass_kernel_spmd(
        nc, in_maps, core_ids=list(range(NCORES)), trace=PROFILE
    )
    LAST_RESULTS = res
    out = np.concatenate([res.results[c]["out"] for c in range(NCORES)], axis=0)
    return out.astype(np.float32)


# revision 13
# speedup vs baseline: 1.3235x; 1.3235x over previous
"""Trainium2 Bass kernel for an 11-stage butterfly linear layer + bias.

Problem: x (16384, 2048) fp32; out[b, :] = B @ x[b, :] + bias where B is the
composition of 11 butterfly stages (strides 1..1024), each an elementwise 2x2
mix of position pairs with learned per-pair coefficients.

Factorization (positions p = blk*128 + w, blk in [0,16), w in [0,128)):
  - Stages 0-6 (strides 1..64) mix within a 128-block -> block-diagonal
    D = diag(D_0..D_15), each 128x128 dense.
  - Stages 7-10 (strides 128..1024) mix across blocks, separately per w ->
    per-w 16x16 matrices C_w.  Grouping q = w8*16 + b over w-group t = w//8
    makes this block-diagonal too (128x128 per group t).

v3 design (vs 185 us baseline / 148 us v2):
  - x pre-transposed on the HOST into per-chunk [pos, block, batch] layout:
    zero TensorE transposes, contiguous input DMA.
  - bf16 output (host casts back to f32): per-core HBM 8 MiB in + 8 MiB out.
  - CH=1024 batch rows per chunk (2 chunks/core): halves the DMA instruction
    count vs CH=512 (dma_start costs ~0.6-2.7us of ring-sequencer time each)
    and doubles permute descriptor size to 2 KiB.
  - Mid permute as 16 per-t SBUF->SBUF DMAs per chunk, alternating SP/ACT
    rings, into per-t Yp tiles (finer deps, less SBUF).
  - MM2 matmuls write STRIDED PSUM APs (out block-major) so the DVE bias-add
    drain reads/writes contiguously (measured 1.2us for the 4D-AP variant).
  - MM1 drains split ACT(3/4) / DVE(1/4); software-pipelined emission keeps
    the PE inside the HAM window.
"""

import sys

import numpy as np

sys.path.insert(0, "/opt/trn_rl_repo")

import concourse.bass as bass  # noqa: E402
import concourse.mybir as mybir  # noqa: E402
import concourse.tile as tile  # noqa: E402
from concourse import bacc  # noqa: E402
from concourse.bass import ds, ts  # noqa: E402
from concourse.bass_utils import run_bass_kernel_spmd  # noqa: E402

N = 2048
LOG_N = 11
NCORES = 8
BATCH = 16384
BPC = BATCH // NCORES  # batch rows per core
P = 128
NB = 16  # number of 128-blocks
CH = 1024  # batch rows per pipeline chunk
CHN = BPC // CH  # chunks per core
HQ = CH // P  # 128-row groups per chunk

WARMUP_MMS = 16  # PE warmup matmuls (N=256) overlapping the first input DMA
STRIDED_PSUM_MM2 = False  # MM2 writes block-major psum -> contiguous DVE add

PROFILE = False
LAST_RESULTS = None

_NC_CACHE = {}


def _emit_body(ctx, tc, aps):
    nc = tc.nc
    x_ap, w1_ap, c2_ap, bb_ap, out_ap = aps
    f32 = mybir.dt.float32
    bf16 = mybir.dt.bfloat16

    const = ctx.enter_context(tc.tile_pool(name="const", bufs=1))
    W1 = const.tile([P, NB * P], bf16)
    C2 = const.tile([P, NB * P], bf16)
    BB = const.tile([P, N], bf16)
    nc.scalar.dma_start(W1[:], w1_ap)
    nc.scalar.dma_start(C2[:], c2_ap)
    nc.scalar.dma_start(BB[:], bb_ap)

    HB = NB // 2  # b-blocks per input half-tile
    xpool = ctx.enter_context(tc.tile_pool(name="xin", bufs=2))
    ypool = ctx.enter_context(tc.tile_pool(name="ymid", bufs=2))
    yppool = ctx.enter_context(tc.tile_pool(name="ypmid", bufs=2 * NB))
    opool = ctx.enter_context(tc.tile_pool(name="oout", bufs=4))
    ps_m1 = ctx.enter_context(tc.tile_pool(name="ps_m1", bufs=2, space="PSUM"))
    ps_m2 = ctx.enter_context(tc.tile_pool(name="ps_m2", bufs=2, space="PSUM"))

    # ---- input DMAs: two half-tiles (8 b-blocks) per chunk ----
    xts = {}
    for c in range(CHN):
        for h in range(2):
            xts[(c, h)] = xpool.tile([P, HB * CH], bf16, name=f"A_{c}_{h}", tag="A")

    def dma_in(c, h):
        nc.sync.dma_start(
            xts[(c, h)][:],
            x_ap[c * P : (c + 1) * P, h * HB * CH : (h + 1) * HB * CH],
        )

    dma_in(0, 0)
    dma_in(0, 1)

    # ---- PE warmup on a memset tile: no DMA dependency, starts immediately ----
    wt = const.tile([P, 2 * P], bf16)
    nc.vector.memset(wt[:], 1.0)
    wps = ps_m1.tile([P, CH], f32, name="warm", tag="pp")
    for i in range(WARMUP_MMS):
        nc.tensor.matmul(
            wps[:, ds(0, 256)], wt[:, ts(0, P)], wt[:], start=True, stop=True
        )

    ypss = {}

    def mm1_block(c):
        """Per b: 2 matmuls (N=512) into a 2-bank PSUM tile + copy drain."""
        Ysb = ypool.tile([P, NB * CH], bf16, name=f"Ysb_{c}", tag="Ysb")
        for b in range(NB):
            A = xts[(c, b // HB)]
            pp = ps_m1.tile([P, CH], f32, name=f"pp_{c}_{b}", tag="pp")
            for i in range(2):
                nc.tensor.matmul(
                    pp[:, ts(i, CH // 2)],
                    W1[:, ts(b, P)],
                    A[:, ds((b % HB) * CH + i * (CH // 2), CH // 2)],
                    start=True,
                    stop=True,
                )
            if b % 4 == 3:
                nc.vector.tensor_copy(Ysb[:, ts(b, CH)], pp[:])
            else:
                nc.scalar.copy(Ysb[:, ts(b, CH)], pp[:])
        # permute: Yp_t[w8*16+b, f] = Ysb[sigma(t,w8), b*CH+f], where the
        # W1 column order sigma(t,w8) = 32*(t//4) + t%4 + 4*w8 spreads each
        # w-group's 8 source partitions over 8 distinct SBUF AXI ports
        # (consecutive partitions share ports 2:1 -> stride-4 maximizes
        # read bandwidth of the gather).
        srcv = Ysb[:].rearrange(
            "(B w r) (b f) -> B r w b f", B=4, w=8, r=4, b=NB, f=CH
        )
        yps = [None] * NB
        # issue order pairs t (even ports, t<8) with t+8 (odd ports) across
        # the two HWDGE rings so concurrent gathers touch disjoint ports
        for i, t in enumerate(x for u in range(8) for x in (u, u + 8)):
            Yp = yppool.tile([P, CH], bf16, name=f"Yp_{c}_{t}", tag="Yp")
            eng = nc.sync if i % 2 == 0 else nc.scalar
            eng.dma_start(Yp[:], srcv[t // 4, t % 4])
            yps[t] = Yp
        ypss[c] = yps

    def mm2_block(c):
        """Per hh: 16 data-stationary matmuls + bias-add drains + DMA out."""
        yps = ypss[c]
        for hh in range(HQ):
            O = opool.tile([P, N], bf16, name=f"O_{c}_{hh}", tag="O")
            for tp in range(2):
                pz = ps_m2.tile([P, 8 * P], f32, name=f"pz_{c}_{hh}_{tp}", tag="pz")
                for j in range(8):
                    t = tp * 8 + j
                    if STRIDED_PSUM_MM2:
                        # col (bo, wo8) -> pz col bo*64 + j*8 + wo8 (block-major)
                        dst = pz[:].rearrange(
                            "p (b t w) -> p t b w", b=16, t=8, w=8
                        )[:, j]
                    else:
                        dst = pz[:, ts(j, P)]
                    nc.tensor.matmul(
                        dst,
                        yps[t][:, ts(hh, P)],
                        C2[:, ts(t, P)],
                        start=True,
                        stop=True,
                    )
                if STRIDED_PSUM_MM2:
                    # O col n = bo*128 + tp*64 + v, v = j*8+wo8 contiguous 64
                    dsto = O[:].rearrange("p (b u) -> p b u", b=16, u=128)[
                        :, :, tp * 64 : (tp + 1) * 64
                    ]
                    src = pz[:].rearrange("p (b v) -> p b v", b=16, v=64)
                    bsrc = BB[:].rearrange("p (b u) -> p b u", b=16, u=128)[
                        :, :, tp * 64 : (tp + 1) * 64
                    ]
                else:
                    dsto = O[:].rearrange("p (b t w) -> p b t w", b=16, t=16, w=8)[
                        :, :, tp * 8 : (tp + 1) * 8, :
                    ]
                    src = pz[:].rearrange("p (t b w) -> p b t w", t=8, b=16, w=8)
                    bsrc = BB[:].rearrange("p (b t w) -> p b t w", b=16, t=16, w=8)[
                        :, :, tp * 8 : (tp + 1) * 8, :
                    ]
                nc.vector.tensor_add(dsto, src, bsrc)
            nc.sync.dma_start(
                out_ap[c * CH + hh * P : c * CH + (hh + 1) * P, :], O[:]
            )

    for c in range(CHN):
        mm1_block(c)
        if c + 1 < CHN:
            dma_in(c + 1, 0)
            dma_in(c + 1, 1)
        if c >= 1:
            mm2_block(c - 1)
    mm2_block(CHN - 1)


def build_nc():
    nc = bacc.Bacc(
        "TRN2",
        target_bir_lowering=False,
        debug=False,
        num_devices=NCORES,
    )
    x_ap = nc.dram_tensor(
        "x", [CHN * P, NB * CH], mybir.dt.bfloat16, kind="ExternalInput"
    ).ap()
    w1_ap = nc.dram_tensor("w1", [P, NB * P], mybir.dt.bfloat16, kind="ExternalInput").ap()
    c2_ap = nc.dram_tensor("c2", [P, NB * P], mybir.dt.bfloat16, kind="ExternalInput").ap()
    bb_ap = nc.dram_tensor("bb", [P, N], mybir.dt.bfloat16, kind="ExternalInput").ap()
    out_ap = nc.dram_tensor("out", [BPC, N], mybir.dt.bfloat16, kind="ExternalOutput").ap()

    from contextlib import ExitStack

    with tile.TileContext(nc) as tc:
        with ExitStack() as ctx:
            _emit_body(ctx, tc, (x_ap, w1_ap, c2_ap, bb_ap, out_ap))
    nc.compile()
    return nc


def _butterfly_apply(tw, X, idx_lo, idx_hi):
    """Apply butterfly stages [idx_lo, idx_hi) to rows of X. tw: (LOG_N, N//2, 2, 2)."""
    out = X
    for idx in range(idx_lo, idx_hi):
        s = 1 << idx
        g = N // (2 * s)
        T = tw[idx].reshape(g, s, 2, 2)
        xr = out.reshape(-1, g, 2, s)
        out = np.einsum("gsij,bgjs->bgis", T, xr).reshape(-1, N)
    return out


def host_weights(twiddle, bias):
    """Build device constants from the twiddle/bias arrays."""
    import ml_dtypes

    tw = np.asarray(twiddle, dtype=np.float64)[0, 0]  # (LOG_N, N//2, 2, 2)
    eye = np.eye(N, dtype=np.float64)
    R1 = _butterfly_apply(tw, eye, 0, 7)  # = D^T, block-diagonal
    R2 = _butterfly_apply(tw, eye, 7, LOG_N)  # = C^T

    # W1 lhsT per block b: lhsT[p, sigma(w)] = D_b[w, p] = R1 block (b, b).
    # sigma(t*8+w8) = 32*(t//4) + t%4 + 4*w8 spreads each w-group over the
    # SBUF AXI ports so the mid permute reads at full fabric rate.
    w = np.arange(P)
    sigma = 32 * (w // 8 // 4) + (w // 8) % 4 + 4 * (w % 8)
    w1 = np.zeros((P, NB * P))
    for b in range(NB):
        w1[:, b * P + sigma] = R1[b * P : (b + 1) * P, b * P : (b + 1) * P]
    # C2 lhsT per w-group t: rows q = w8*16+b (mid pos), cols j = bo*8+wo8 (out pos)
    c2 = np.zeros((P, NB * P))
    q = np.arange(P)
    for t in range(NB):
        pm = (q % 16) * P + t * 8 + (q // 16)
        pn = (q // 8) * P + t * 8 + (q % 8)
        c2[:, t * P : (t + 1) * P] = R2[np.ix_(pm, pn)]
    bb = np.broadcast_to(np.asarray(bias, dtype=np.float64)[None, :], (P, N))
    return (
        np.ascontiguousarray(w1.astype(ml_dtypes.bfloat16)),
        np.ascontiguousarray(c2.astype(ml_dtypes.bfloat16)),
        np.ascontiguousarray(bb.astype(ml_dtypes.bfloat16)),
    )


def host_x(x):
    """bf16-cast + per-core chunked transpose: [c][fc][p][b][f] layout."""
    import ml_dtypes

    xb = np.asarray(x).astype(ml_dtypes.bfloat16)
    # rows = c*2048 + fc*CH + f; cols = b*128 + p
    xr = xb.reshape(NCORES, CHN, CH, NB, P).transpose(0, 1, 4, 3, 2)
    return np.ascontiguousarray(xr)  # (8, CHN, 128, 16, CH)


def kernel(x, twiddle, bias):
    global LAST_RESULTS

    assert x.shape == (BATCH, N), x.shape

    if "nc" not in _NC_CACHE:
        _NC_CACHE["nc"] = build_nc()
    nc = _NC_CACHE["nc"]

    w1, c2, bb = host_weights(twiddle, bias)
    xr = host_x(x)
    in_maps = [
        {
            "x": xr[c].reshape(CHN * P, NB * CH),
            "w1": w1,
            "c2": c2,
            "bb": bb,
        }
        for c in range(NCORES)
    ]
    res = run_bass_kernel_spmd(
        nc, in_maps, core_ids=list(range(NCORES)), trace=PROFILE
    )
    LAST_RESULTS = res
    out = np.concatenate([res.results[c]["out"] for c in range(NCORES)], axis=0)
    return out.astype(np.float32)
